# revision 1
# baseline (speedup 1.0000x reference)
"""EnergyTransformerLayer on 8 Trainium2 NeuronCores (Bass/Tile).

Sharding (per spec hint): heads are sharded across the 8 cores (2 heads each)
for the 5-step energy-descent loop; Q_opt is exchanged with an AllToAll before
the Wo projection; the Wo projection + residual + FFN are sharded by target
rows (128 rows per core), so the host assembles the final output by
concatenating per-core row blocks.

All layout preparation (transposes, dtype casts, per-core slicing) happens on
the host in numpy; all math (tanh, projections, energy loop, softmax-descent,
FFN) runs on device.

Softmax-free descent step (per head, all in transposed layout):
    scoresT[k, t] = sum_z K[k, z] q[t, z]            (MM1, z=64 row-packed x2)
    ex = exp(beta * scoresT)                         (ACT, PSUM->SBUF bf16)
    upd = [K | 1/step]^T @ ex                        (MM2: rows 0-63 = num,
                                                      rows 64-127 = rowsum/step)
    qT += num * reciprocal(rowsum/step)              (DVE)
"""
import numpy as np
import ml_dtypes

import concourse.bass as bass
import concourse.mybir as mybir
import concourse.tile as tile
from concourse import bacc
from concourse.bass_utils import run_bass_kernel_spmd
from concourse.masks import make_identity

dt = mybir.dt
AF = mybir.ActivationFunctionType

N_CORES = 8
EMBED = 1024
N_HEADS = 16
HD = 64
HIDDEN = 4096
N_CTX = 2048
N_TGT = 1024
STEPS = 5
BETA = 1.0 / 8.0          # BETA / sqrt(HD)
INV_STEP = 10.0           # 1 / STEP_SIZE, folded into the ones-block of K_aug

HPC = N_HEADS // N_CORES  # heads per core = 2
TPC = N_TGT // N_CORES    # target rows per core = 128

BF = dt.bfloat16
F32 = dt.float32

# swappable for simulation (CoreSim implements no gelu variant)
GELU_FN = AF.Gelu_apprx_tanh
# split ramp input DMAs into halves for queue fan-out (A/B toggle)
SPLIT_IN_DMA = True

DC = EMBED // 128     # 8 d-chunks
KC = N_CTX // 128     # 16 k-chunks
HC = HIDDEN // 128    # 32 hidden-chunks


def build_kernel(replicas: int = 1, no_collective: bool = False,
                 loop_n: int = 1, gate_weights: bool = True,
                 skip_tail: bool = False):
    """Build the SPMD Bacc program (same NEFF on all 8 cores).

    no_collective=True replaces the AllToAll with a local DRAM copy — only
    for timing/timeline analysis (the A2A cost is excluded; output is wrong
    in the t-block mixing sense but numerically representative).
    loop_n>1 wraps the body in a hardware For_i loop for precise timing.
    """
    nc = bacc.Bacc("TRN2", target_bir_lowering=False, debug=False,
                   num_devices=N_CORES)

    ctxT_d = nc.dram_tensor("ctxT", [EMBED, N_CTX], BF, kind="ExternalInput")
    tgtT_d = nc.dram_tensor("tgtT", [EMBED, N_TGT], BF, kind="ExternalInput")
    tgt_rows_d = nc.dram_tensor("tgt_rows", [TPC, EMBED], F32, kind="ExternalInput")
    wqkT_d = nc.dram_tensor("wqkT", [EMBED, 2 * HPC * HD], BF, kind="ExternalInput")
    woT_d = nc.dram_tensor("woT", [EMBED, EMBED], BF, kind="ExternalInput")
    w1T_d = nc.dram_tensor("w1T", [EMBED, HIDDEN], BF, kind="ExternalInput")
    w2T_d = nc.dram_tensor("w2T", [HIDDEN, EMBED], BF, kind="ExternalInput")
    alphas_d = nc.dram_tensor("alphas", [128, 2], F32, kind="ExternalInput")
    out_d = nc.dram_tensor("out_rows", [TPC, EMBED], F32, kind="ExternalOutput")

    with tile.TileContext(nc) as tc:
        with (
            tc.tile_pool(name="const", bufs=1) as cpool,
            tc.tile_pool(name="persist", bufs=1) as pp,
            tc.tile_pool(name="wts", bufs=1) as wp,
            tc.tile_pool(name="stream", bufs=3) as sp,
            tc.tile_pool(name="work", bufs=1) as wk,
            tc.tile_pool(name="psA", bufs=3, space="PSUM") as psA,  # [128,1024]f32: 2 banks
            tc.tile_pool(name="psB", bufs=2, space="PSUM") as psB,  # [128,512]f32: 1 bank
            tc.tile_pool(name="dram", bufs=1, space="DRAM") as dp,
        ):
            alphas = cpool.tile([128, 2], F32)
            nc.sync.dma_start(out=alphas[:], in_=alphas_d[:])
            ident = cpool.tile([128, 128], BF)
            make_identity(nc, ident[:])

            wqkT = cpool.tile([128, DC * 256], BF)        # [d-chunk | wq128 wk128]
            nc.sync.dma_start(
                out=wqkT[:].rearrange("p (a f) -> p a f", a=DC),
                in_=wqkT_d.rearrange("(a p) f -> p a f", p=128),
            )
            woT_sb = wp.tile([128, DC * EMBED], BF)       # [d-chunk | e]

            def body(rep):
                # ------------- phase 1+2: tnorm, K / q projections ----------
                KT = pp.tile([128, N_CTX], BF, tag="KT", name=f"KT{rep}")
                Kaug = pp.tile([128, KC * 2 * 128], BF, tag="Kaug",
                               name=f"Kaug{rep}")
                nc.gpsimd.memset(Kaug[:], INV_STEP)
                qT = pp.tile([128, N_TGT], F32, tag="qT", name=f"qT{rep}")

                kps = [psA.tile([128, 1024], F32, tag="psA", name=f"kps{rep}_{i}")
                       for i in range(2)]
                qps = psA.tile([128, 1024], F32, tag="psA", name=f"qps{rep}")
                last_in_dma = None
                nsp = 2 if SPLIT_IN_DMA else 1
                for d in range(DC):
                    # split each chunk for more DMA queue fan-out (if enabled)
                    ctx_t = sp.tile([128, N_CTX], BF, tag="ctx", name=f"ctx{rep}_{d}")
                    cw = N_CTX // nsp
                    for hh in range(nsp):
                        last_in_dma = nc.sync.dma_start(
                            out=ctx_t[:, hh * cw:(hh + 1) * cw],
                            in_=ctxT_d.rearrange("(a p) k -> p a k", p=128)[
                                :, d, hh * cw:(hh + 1) * cw],
                        )
                    tgt_t = sp.tile([128, N_TGT], BF, tag="tgt", name=f"tgt{rep}_{d}")
                    tw = N_TGT // nsp
                    for hh in range(nsp):
                        nc.sync.dma_start(
                            out=tgt_t[:, hh * tw:(hh + 1) * tw],
                            in_=tgtT_d.rearrange("(a p) t -> p a t", p=128)[
                                :, d, hh * tw:(hh + 1) * tw],
                        )
                    tn_t = sp.tile([128, N_TGT], BF, tag="tn", name=f"tn{rep}_{d}")
                    nc.scalar.activation(tn_t[:], tgt_t[:], AF.Tanh,
                                         scale=alphas[:, 0:1])
                    wq = wqkT[:, d * 256:d * 256 + 128]
                    wkk = wqkT[:, d * 256 + 128:d * 256 + 256]
                    first, last = d == 0, d == DC - 1
                    for kcol in range(4):
                        nc.tensor.matmul(
                            kps[kcol // 2][:, (kcol % 2) * 512:(kcol % 2 + 1) * 512],
                            wkk, ctx_t[:, kcol * 512:(kcol + 1) * 512],
                            start=first, stop=last)
                    for tcol in range(2):
                        nc.tensor.matmul(
                            qps[:, tcol * 512:(tcol + 1) * 512],
                            wq, tn_t[:, tcol * 512:(tcol + 1) * 512],
                            start=first, stop=last)
                for i in range(2):
                    nc.vector.tensor_copy(
                        KT[:, i * 1024:(i + 1) * 1024], kps[i][:])
                nc.vector.tensor_copy(qT[:], qps[:])

                # transpose K_hT -> K_aug blocks ([k, z] layout per head)
                for kc in range(KC):
                    ktp = psB.tile([128, 128], BF, tag="psB", name=f"ktp{rep}_{kc}")
                    nc.tensor.transpose(ktp[:], KT[:, kc * 128:(kc + 1) * 128],
                                        ident[:])
                    base = kc * 256
                    # one strided copy fills both heads' K blocks (the 10.0
                    # ones-blocks in between were memset already)
                    nc.vector.tensor_copy(
                        Kaug[:, base:base + 256].rearrange(
                            "p (h f) -> p h f", f=128)[:, :, 0:64],
                        ktp[:].rearrange("p (h f) -> p h f", f=64),
                    )

                # FFN / Wo weight streaming: emit DMAs early so the queues
                # stay busy during the descent loop, but gate them behind the
                # last input DMA so the ramp (ctx/tgt) isn't contended.
                from concourse.tile import add_dep_helper

                gate = last_in_dma.ins
                w1cs, w2cs = [], []
                for a in range(DC):
                    wd = nc.sync.dma_start(
                        out=woT_sb[:, a * EMBED:(a + 1) * EMBED],
                        in_=woT_d.rearrange("(a p) e -> p a e", p=128)[:, a, :],
                    )
                    if gate_weights:
                        add_dep_helper(wd.ins, gate, sync=True,
                                       reason="after ramp")
                for q in range(4):
                    w1c = wp.tile([128, DC * 1024], BF, tag="w1s", bufs=2,
                                  name=f"w1c{rep}_{q}")
                    for a in range(DC):
                        wd = nc.sync.dma_start(
                            out=w1c[:, a * 1024:(a + 1) * 1024],
                            in_=w1T_d.rearrange("(a p) h -> p a h", p=128)[
                                :, a, q * 1024:(q + 1) * 1024],
                        )
                        if gate_weights:
                            add_dep_helper(wd.ins, gate, sync=True,
                                           reason="after ramp")
                    w1cs.append(w1c)
                for q in range(4):
                    w2c = wp.tile([128, 8 * EMBED], BF, tag="w2s", bufs=2,
                                  name=f"w2c{rep}_{q}")
                    for j in range(8):
                        hc = q * 8 + j
                        wd = nc.sync.dma_start(
                            out=w2c[:, j * EMBED:(j + 1) * EMBED],
                            in_=w2T_d.rearrange("(a p) e -> p a e", p=128)[:, hc, :],
                        )
                        if gate_weights:
                            add_dep_helper(wd.ins, gate, sync=True,
                                           reason="after ramp")
                    w2cs.append(w2c)

                # ------------- phase 3: 5-step energy descent ---------------
                qbf = {}
                for th in range(2):
                    tsl = slice(th * 512, (th + 1) * 512)
                    b = wk.tile([128, 512], BF, tag=f"qbf{th}", bufs=2,
                                name=f"qbf{rep}_init{th}")
                    nc.vector.tensor_copy(b[:], qT[:, tsl])
                    qbf[th] = b
                for step in range(STEPS):
                    for th in range(2):
                        tsl = slice(th * 512, (th + 1) * 512)
                        upd = [psB.tile([128, 512], F32, tag="psB",
                                        name=f"upd{rep}_{step}_{th}_{h}")
                               for h in range(2)]
                        for kc in range(KC):
                            sc = psA.tile([128, 1024], F32, tag="psA",
                                          name=f"sc{rep}_{step}_{th}_{kc}")
                            for h in range(2):
                                nc.tensor.matmul(
                                    sc[:, h * 512:(h + 1) * 512],
                                    KT[h * 64:(h + 1) * 64, kc * 128:(kc + 1) * 128],
                                    qbf[th][h * 64:(h + 1) * 64, :],
                                    start=True, stop=True,
                                )
                            ex = wk.tile([128, 1024], BF, tag="ex", bufs=6,
                                         name=f"ex{rep}_{step}_{th}_{kc}")
                            nc.scalar.activation(ex[:], sc[:], AF.Exp, scale=BETA)
                            for h in range(2):
                                nc.tensor.matmul(
                                    upd[h][:],
                                    Kaug[:, kc * 256 + h * 128:kc * 256 + (h + 1) * 128],
                                    ex[:, h * 512:(h + 1) * 512],
                                    start=(kc == 0), stop=(kc == KC - 1),
                                )
                        for h in range(2):
                            rec = wk.tile([128, 512], F32, tag="rec", bufs=2,
                                          name=f"rec{rep}_{step}_{th}_{h}")
                            nc.vector.reciprocal(rec[64:128, :], upd[h][64:128, :])
                            dq = wk.tile([128, 512], F32, tag="dq", bufs=2,
                                         name=f"dq{rep}_{step}_{th}_{h}")
                            hsl = slice(h * 64, (h + 1) * 64)
                            nc.vector.tensor_tensor(
                                dq[hsl, :], upd[h][0:64, :], rec[64:128, :],
                                mybir.AluOpType.mult,
                            )
                            nc.vector.tensor_tensor(
                                qT[hsl, tsl], qT[hsl, tsl], dq[hsl, :],
                                mybir.AluOpType.add,
                            )
                        if step < STEPS - 1:
                            b = wk.tile([128, 512], BF, tag=f"qbf{th}", bufs=2,
                                        name=f"qbf{rep}_{step}_{th}")
                            nc.vector.tensor_copy(b[:], qT[:, tsl])
                            qbf[th] = b

                if skip_tail:
                    out_sb0 = wk.tile([128, EMBED], F32, tag="out_sb",
                                      name=f"outq{rep}")
                    nc.vector.tensor_copy(out_sb0[:], qT[:])
                    nc.sync.dma_start(out=out_d[:], in_=out_sb0[:])
                    return

                # ------------- phase 4: AllToAll on Q -----------------------
                # qT is [128 (my 2 heads' z), 1024 (all t)]. Stack the 8
                # t-blocks on the DRAM partition axis; after AllToAll shard j
                # holds rank j's heads at MY t-block => QT[:, my t cols] with
                # global head ordering. Identical program on every core.
                # cast q to bf16 BEFORE the collective: it is consumed in
                # bf16 by the Wo matmul anyway, and this halves A2A bytes.
                qfin = wk.tile([128, N_TGT], BF, tag="qfin", name=f"qfin{rep}")
                nc.vector.tensor_copy(qfin[:], qT[:])
                q_loc = dp.tile([N_CORES * 128, TPC], BF, name=f"qloc{rep}")
                q_ex = dp.tile([N_CORES * 128, TPC], BF, name=f"qex{rep}")
                nc.sync.dma_start(
                    out=q_loc[:].rearrange("(j p) t -> p j t", p=128),
                    in_=qfin[:].rearrange("p (j t) -> p j t", j=N_CORES),
                )
                if no_collective:
                    nc.sync.dma_start(out=q_ex[:], in_=q_loc[:])
                else:
                    nc.gpsimd.collective_compute(
                        "AllToAll",
                        mybir.AluOpType.bypass,
                        replica_groups=[list(range(N_CORES))],
                        ins=[q_loc[:]],
                        outs=[q_ex[:]],
                    )
                qto = wk.tile([128, DC * TPC], BF, tag="qto", name=f"qto{rep}")
                nc.sync.dma_start(
                    out=qto[:].rearrange("p (a t) -> p a t", a=DC),
                    in_=q_ex[:].rearrange("(a p) t -> p a t", p=128),
                )

                # ------------- phase 5: Wo projection + residual ------------
                atn = psA.tile([128, 1024], F32, tag="psA", name=f"atn{rep}")
                for a in range(DC):
                    for ecol in range(2):
                        nc.tensor.matmul(
                            atn[:, ecol * 512:(ecol + 1) * 512],
                            qto[:, a * TPC:(a + 1) * TPC],
                            woT_sb[:, a * EMBED + ecol * 512:
                                   a * EMBED + (ecol + 1) * 512],
                            start=(a == 0), stop=(a == DC - 1),
                        )
                tgt_r = wk.tile([128, EMBED], F32, tag="tgt_r", name=f"tgtr{rep}")
                nc.sync.dma_start(out=tgt_r[:], in_=tgt_rows_d[:])
                t2 = pp.tile([128, EMBED], F32, tag="t2", name=f"t2{rep}")
                nc.vector.tensor_tensor(t2[:], tgt_r[:], atn[:],
                                        mybir.AluOpType.add)
                t2n = wk.tile([128, EMBED], BF, tag="t2n", name=f"t2n{rep}")
                nc.scalar.activation(t2n[:], t2[:], AF.Tanh, scale=alphas[:, 1:2])
                t2T = wk.tile([128, DC * TPC], BF, tag="t2T", name=f"t2T{rep}")
                for d in range(DC):
                    tp = psB.tile([128, 128], BF, tag="psB", name=f"t2tp{rep}_{d}")
                    nc.tensor.transpose(tp[:], t2n[:, d * 128:(d + 1) * 128],
                                        ident[:])
                    nc.vector.tensor_copy(t2T[:, d * TPC:(d + 1) * TPC], tp[:])

                # ------------- phase 6: FFN ---------------------------------
                # phase 6a: H = t2 @ W1.T in quarters of hidden (streamed W1T)
                # quarter-pairs with d outer: the stationary t2T[d] is loaded
                # once per d and reused across 4 matmuls (both quarters of
                # the pair) instead of reloaded per (q, d).
                G = wk.tile([128, HIDDEN], BF, tag="G", name=f"G{rep}")
                for qp in range(2):
                    hp2 = [psA.tile([128, 1024], F32, tag="psA",
                                    name=f"hps{rep}_{qp}_{i}") for i in range(2)]
                    for d in range(DC):
                        for i in range(2):
                            q = qp * 2 + i
                            w1c = w1cs[q]
                            for j in range(2):
                                nc.tensor.matmul(
                                    hp2[i][:, j * 512:(j + 1) * 512],
                                    t2T[:, d * TPC:(d + 1) * TPC],
                                    w1c[:, d * 1024 + j * 512:
                                        d * 1024 + (j + 1) * 512],
                                    start=(d == 0), stop=(d == DC - 1),
                                )
                    for i in range(2):
                        q = qp * 2 + i
                        nc.scalar.activation(
                            G[:, q * 1024:(q + 1) * 1024], hp2[i][:], GELU_FN
                        )
                # phase 6b: transpose G -> GT
                GT = wk.tile([128, HIDDEN], BF, tag="GT", name=f"GT{rep}")
                for hc in range(HC):
                    gp = psB.tile([128, 128], BF, tag="psB", name=f"gtp{rep}_{hc}")
                    nc.tensor.transpose(gp[:], G[:, hc * 128:(hc + 1) * 128],
                                        ident[:])
                    nc.vector.tensor_copy(GT[:, hc * 128:(hc + 1) * 128], gp[:])
                # phase 6c: ffn = G @ W2.T (streamed W2T, 8 hid-chunks per dma)
                fps = psA.tile([128, 1024], F32, tag="psA", name=f"fps{rep}")
                for q in range(4):
                    w2c = w2cs[q]
                    for j in range(8):
                        hc = q * 8 + j
                        for ecol in range(2):
                            nc.tensor.matmul(
                                fps[:, ecol * 512:(ecol + 1) * 512],
                                GT[:, hc * 128:(hc + 1) * 128],
                                w2c[:, j * EMBED + ecol * 512:
                                    j * EMBED + (ecol + 1) * 512],
                                start=(hc == 0), stop=(hc == HC - 1),
                            )
                out_sb = wk.tile([128, EMBED], F32, tag="out_sb", name=f"out{rep}")
                nc.vector.tensor_tensor(out_sb[:], t2[:], fps[:],
                                        mybir.AluOpType.add)
                nc.sync.dma_start(out=out_d[:], in_=out_sb[:])

            if loop_n > 1:
                assert no_collective and replicas == 1
                with tc.For_i(0, loop_n, 1):
                    body(0)
            else:
                for rep in range(replicas):
                    body(rep)

    nc.compile()
    return nc


def prepare_inputs(context, target, Wq, Wk, Wo, W1, W2, alpha1, alpha2):
    """Per-core host-side layout prep. Returns list of 8 in_maps."""
    bf = ml_dtypes.bfloat16
    context = np.asarray(context, np.float32)
    target = np.asarray(target, np.float32)
    ctxT = np.ascontiguousarray(context.T).astype(bf)            # [1024, 2048]
    tgtT = np.ascontiguousarray(target.T).astype(np.float32)     # [1024, 1024]
    woT = np.ascontiguousarray(np.asarray(Wo, np.float32).T).astype(bf)
    w1T = np.ascontiguousarray(np.asarray(W1, np.float32).T).astype(bf)
    w2T = np.ascontiguousarray(np.asarray(W2, np.float32).T).astype(bf)
    alphas = np.zeros((128, 2), np.float32)
    alphas[:, 0] = np.float32(np.asarray(alpha1).reshape(-1)[0])
    alphas[:, 1] = np.float32(np.asarray(alpha2).reshape(-1)[0])
    Wq = np.asarray(Wq, np.float32)
    Wk = np.asarray(Wk, np.float32)

    tgtT = tgtT.astype(bf)
    in_maps = []
    for c in range(N_CORES):
        hs = slice(c * HPC, (c + 1) * HPC)
        wq = Wq[hs].reshape(HPC * HD, EMBED)
        wkk = Wk[hs].reshape(HPC * HD, EMBED)
        wqkT = np.concatenate(
            [np.ascontiguousarray(wq.T), np.ascontiguousarray(wkk.T)], axis=1
        ).astype(bf)                                             # [1024, 256]
        in_maps.append({
            "ctxT": ctxT,
            "tgtT": tgtT,
            "tgt_rows": np.ascontiguousarray(
                target[c * TPC:(c + 1) * TPC]).astype(np.float32),
            "wqkT": wqkT,
            "woT": woT,
            "w1T": w1T,
            "w2T": w2T,
            "alphas": alphas,
        })
    return in_maps


def kernel(context, target, Wq, Wk, Wo, W1, W2, alpha1, alpha2):
    in_maps = prepare_inputs(context, target, Wq, Wk, Wo, W1, W2,
                             alpha1, alpha2)
    nc = build_kernel()
    res = run_bass_kernel_spmd(nc, in_maps, list(range(N_CORES)))
    out = np.concatenate(
        [res.results[c]["out_rows"] for c in range(N_CORES)], axis=0
    )
    return out.astype(np.float32)



# revision 3
# speedup vs baseline: 1.2562x; 1.2562x over previous
"""EnergyTransformerLayer on 8 Trainium2 NeuronCores (Bass/Tile).

Sharding (per spec hint): heads are sharded across the 8 cores (2 heads each)
for the 5-step energy-descent loop; Q_opt is exchanged with an AllToAll before
the Wo projection; the Wo projection + residual + FFN are sharded by target
rows (128 rows per core), so the host assembles the final output by
concatenating per-core row blocks.

Key optimizations over the v1 baseline:
  - The energy loop is jointly scheduled across four engines: PE does the
    score/update matmuls (software-pipelined with a 2-chunk lag so PE never
    waits on exp), ACT computes exp for ~23/32 chunks per step, DVE computes
    exp for the rest via a 2-instruction custom op ((1+t/1024)^1024 squaring
    chain - scores are tightly centered so the approximation error is ~1e-6),
    and the Pool/GPSIMD queue does the q-update elementwise tail.
  - Reciprocal uses the fast approximate custom DVE op.
  - The FFN computes H^T = W1 @ t2^T per 128-hidden-chunk, applies gelu
    straight out of PSUM, and feeds W2 immediately - no G transpose barrier.
  - Wo projection / residual / tanh / transposes are split in halves to
    pipeline, and the A2A staging DMA is issued per t-half as the last
    descent step retires.

Softmax-free descent step (per head, transposed layout):
    scoresT[k, t] = sum_z K[k, z] q[t, z]            (MM1, z=64 row-packed x2)
    ex = exp(beta * scoresT)                         (ACT or DVE, PSUM->SBUF)
    upd = [K | 1/step]^T @ ex                        (MM2: rows 0-63 = num,
                                                      rows 64-127 = rowsum/step)
    qT += num * reciprocal(rowsum/step)              (DVE recip + Pool mul/add)
"""
import numpy as np
import ml_dtypes

import concourse.bass as bass
import concourse.mybir as mybir
import concourse.tile as tile
from concourse import bacc
from concourse.bass_utils import run_bass_kernel_spmd
from concourse.masks import make_identity

dt = mybir.dt
AF = mybir.ActivationFunctionType

N_CORES = 8
EMBED = 1024
N_HEADS = 16
HD = 64
HIDDEN = 4096
N_CTX = 2048
N_TGT = 1024
STEPS = 5
BETA = 1.0 / 8.0          # BETA / sqrt(HD)
INV_STEP = 10.0           # 1 / STEP_SIZE, folded into the ones-block of K_aug

HPC = N_HEADS // N_CORES  # heads per core = 2
TPC = N_TGT // N_CORES    # target rows per core = 128

BF = dt.bfloat16
F32 = dt.float32

# swappable for simulation (CoreSim implements no gelu variant)
GELU_FN = AF.Gelu_apprx_tanh
SPLIT_IN_DMA = True

DC = EMBED // 128     # 8 d-chunks
KC = N_CTX // 128     # 16 k-chunks
HC = HIDDEN // 128    # 32 hidden-chunks

MM2_LAG = 2           # software pipeline: MM2(kc) issued after MM1(kc+LAG)
# k-chunks whose exp runs on DVE (rest on ACT), per t-half
DVE_CHUNKS = {0: (6, 9, 12, 14), 1: (5, 8, 10, 12, 14)}
EXP_N = 1024.0        # exp(t) ~ (1 + t/EXP_N)^EXP_N

# ---------------------------------------------------------------------------
# Custom DVE ops: exp via squaring chain, 2 instructions.
#   A: v = (1 + x*C0)^32      (C0 = beta/EXP_N)
#   B: out = v^32             (=> (1+x*C0)^1024)
# ---------------------------------------------------------------------------


def _register_exp_ops():
    from concourse.dve_spec import Spec, Src0, One, C0, lower
    from concourse.dve_ops import (
        DveOp, OPS, CUSTOM_DVE_SPECS, _SUB_OPCODE_FOR_NAME,
        _CUSTOM_DVE_ROW_BASE, has_src1,
    )
    from concourse.dve_uop import DveOpSpec
    from concourse.dve_table_gen import dve_ver_for

    if "EXP_SQCHAIN_A_ANT" in _SUB_OPCODE_FOR_NAME:
        from concourse import dve_ops
        return dve_ops.EXP_SQCHAIN_A_ANT, dve_ops.EXP_SQCHAIN_B_ANT

    u = Src0 * C0 + One
    for _ in range(5):
        u = u * u

    def ref_a(in0, in1, c0, c1, c2):
        v = (np.float32(1.0)
             + in0.astype(np.float32) * np.asarray(c0, np.float32))
        v = v.astype(np.float32)
        for _ in range(5):
            v = (v * v).astype(np.float32)
        return v

    w = Src0 * Src0
    for _ in range(4):
        w = w * w

    def ref_b(in0, in1, c0, c1, c2):
        v = in0.astype(np.float32)
        for _ in range(5):
            v = (v * v).astype(np.float32)
        return v

    ops = []
    for name, spec in [
        ("EXP_SQCHAIN_A_ANT", Spec(body=u, reference=ref_a)),
        ("EXP_SQCHAIN_B_ANT", Spec(body=w, reference=ref_b)),
    ]:
        row = _CUSTOM_DVE_ROW_BASE + len(OPS)
        assert row < 0x20, "custom-DVE opcode rows exhausted"
        op = DveOp(name, spec, subdim=False, uops_sha={})
        # pin the sha self-consistently (computed from this process's lower())
        for ver in ("v3", "v4"):
            try:
                lowered = DveOpSpec(
                    name=name, opcode=row, uops=lower(spec, ver=ver),
                    rd1_en=has_src1(spec),
                )
                op.uops_sha[ver] = lowered.sha(ver)
            except Exception:
                pass
        OPS.append(op)
        CUSTOM_DVE_SPECS[name] = spec
        _SUB_OPCODE_FOR_NAME[name] = row
        import concourse.dve_ops as dve_ops_mod
        setattr(dve_ops_mod, name, op)
        ops.append(op)
    return ops[0], ops[1]


EXP_A, EXP_B = _register_exp_ops()


def build_kernel(replicas: int = 1, no_collective: bool = False,
                 loop_n: int = 1, gate_weights: bool = True,
                 skip_tail: bool = False):
    """Build the SPMD Bacc program (same NEFF on all 8 cores).

    no_collective=True replaces the AllToAll with a local DRAM copy - only
    for timing/timeline analysis. loop_n>1 wraps the body in a hardware
    For_i loop for precise slope timing.
    """
    nc = bacc.Bacc("TRN2", target_bir_lowering=False, debug=False,
                   num_devices=N_CORES)

    ctxT_d = nc.dram_tensor("ctxT", [EMBED, N_CTX], BF, kind="ExternalInput")
    tgtT_d = nc.dram_tensor("tgtT", [EMBED, N_TGT], BF, kind="ExternalInput")
    tgt_rows_d = nc.dram_tensor("tgt_rows", [TPC, EMBED], F32, kind="ExternalInput")
    wqkT_d = nc.dram_tensor("wqkT", [EMBED, 2 * HPC * HD], BF, kind="ExternalInput")
    woT_d = nc.dram_tensor("woT", [EMBED, EMBED], BF, kind="ExternalInput")
    w1T_d = nc.dram_tensor("w1T", [EMBED, HIDDEN], BF, kind="ExternalInput")
    w2T_d = nc.dram_tensor("w2T", [HIDDEN, EMBED], BF, kind="ExternalInput")
    alphas_d = nc.dram_tensor("alphas", [128, 2], F32, kind="ExternalInput")
    out_d = nc.dram_tensor("out_rows", [TPC, EMBED], F32, kind="ExternalOutput")

    with tile.TileContext(nc) as tc:
        with (
            tc.tile_pool(name="const", bufs=1) as cpool,
            tc.tile_pool(name="persist", bufs=1) as pp,
            tc.tile_pool(name="wts", bufs=1) as wp,
            tc.tile_pool(name="stream", bufs=3) as sp,
            tc.tile_pool(name="work", bufs=1) as wk,
            tc.tile_pool(name="psA", bufs=3, space="PSUM") as psA,  # [128,1024]f32: 2 banks
            tc.tile_pool(name="psB", bufs=2, space="PSUM") as psB,  # [128,512]f32: 1 bank
            tc.tile_pool(name="dram", bufs=1, space="DRAM") as dp,
        ):
            alphas = cpool.tile([128, 2], F32)
            nc.sync.dma_start(out=alphas[:], in_=alphas_d[:])
            ident = cpool.tile([128, 128], BF)
            make_identity(nc, ident[:])

            wqkT = cpool.tile([128, DC * 256], BF)        # [d-chunk | wq128 wk128]
            nc.sync.dma_start(
                out=wqkT[:].rearrange("p (a f) -> p a f", a=DC),
                in_=wqkT_d.rearrange("(a p) f -> p a f", p=128),
            )
            woT_sb = wp.tile([128, DC * EMBED], BF)       # [d-chunk | e]

            def body(rep):
                # ------------- phase 1+2: tnorm, K / q projections ----------
                KT = pp.tile([128, N_CTX], BF, tag="KT", name=f"KT{rep}")
                Kaug = pp.tile([128, KC * 2 * 128], BF, tag="Kaug",
                               name=f"Kaug{rep}")
                nc.gpsimd.memset(Kaug[:], INV_STEP)
                qT = pp.tile([128, N_TGT], F32, tag="qT", name=f"qT{rep}")

                kps = [psA.tile([128, 1024], F32, tag="psA", name=f"kps{rep}_{i}")
                       for i in range(2)]
                qps = psA.tile([128, 1024], F32, tag="psA", name=f"qps{rep}")
                last_in_dma = None
                nsp = 2 if SPLIT_IN_DMA else 1
                for d in range(DC):
                    ctx_t = sp.tile([128, N_CTX], BF, tag="ctx", name=f"ctx{rep}_{d}")
                    cw = N_CTX // nsp
                    for hh in range(nsp):
                        last_in_dma = nc.sync.dma_start(
                            out=ctx_t[:, hh * cw:(hh + 1) * cw],
                            in_=ctxT_d.rearrange("(a p) k -> p a k", p=128)[
                                :, d, hh * cw:(hh + 1) * cw],
                        )
                    tgt_t = sp.tile([128, N_TGT], BF, tag="tgt", name=f"tgt{rep}_{d}")
                    tw = N_TGT // nsp
                    for hh in range(nsp):
                        nc.sync.dma_start(
                            out=tgt_t[:, hh * tw:(hh + 1) * tw],
                            in_=tgtT_d.rearrange("(a p) t -> p a t", p=128)[
                                :, d, hh * tw:(hh + 1) * tw],
                        )
                    tn_t = sp.tile([128, N_TGT], BF, tag="tn", name=f"tn{rep}_{d}")
                    nc.scalar.activation(tn_t[:], tgt_t[:], AF.Tanh,
                                         scale=alphas[:, 0:1])
                    wq = wqkT[:, d * 256:d * 256 + 128]
                    wkk = wqkT[:, d * 256 + 128:d * 256 + 256]
                    first, last = d == 0, d == DC - 1
                    for kcol in range(4):
                        nc.tensor.matmul(
                            kps[kcol // 2][:, (kcol % 2) * 512:(kcol % 2 + 1) * 512],
                            wkk, ctx_t[:, kcol * 512:(kcol + 1) * 512],
                            start=first, stop=last)
                    for tcol in range(2):
                        nc.tensor.matmul(
                            qps[:, tcol * 512:(tcol + 1) * 512],
                            wq, tn_t[:, tcol * 512:(tcol + 1) * 512],
                            start=first, stop=last)
                for i in range(2):
                    nc.vector.tensor_copy(
                        KT[:, i * 1024:(i + 1) * 1024], kps[i][:])
                nc.vector.tensor_copy(qT[:], qps[:])

                # transpose K_hT -> K_aug blocks ([k, z] layout per head)
                for kc in range(KC):
                    ktp = psB.tile([128, 128], BF, tag="psB", name=f"ktp{rep}_{kc}")
                    nc.tensor.transpose(ktp[:], KT[:, kc * 128:(kc + 1) * 128],
                                        ident[:])
                    base = kc * 256
                    nc.gpsimd.tensor_copy(
                        Kaug[:, base:base + 256].rearrange(
                            "p (h f) -> p h f", f=128)[:, :, 0:64],
                        ktp[:].rearrange("p (h f) -> p h f", f=64),
                    )

                # FFN / Wo weight streaming: emit DMAs early so the queues
                # stay busy during the descent loop, gated behind the ramp.
                from concourse.tile import add_dep_helper

                gate = last_in_dma.ins
                w1cs, w2cs = [], []
                for a in range(DC):
                    wd = nc.sync.dma_start(
                        out=woT_sb[:, a * EMBED:(a + 1) * EMBED],
                        in_=woT_d.rearrange("(a p) e -> p a e", p=128)[:, a, :],
                    )
                    if gate_weights:
                        add_dep_helper(wd.ins, gate, sync=True,
                                       reason="after ramp")
                for q in range(4):
                    w1c = wp.tile([128, DC * 1024], BF, tag="w1s", bufs=2,
                                  name=f"w1c{rep}_{q}")
                    for a in range(DC):
                        wd = nc.sync.dma_start(
                            out=w1c[:, a * 1024:(a + 1) * 1024],
                            in_=w1T_d.rearrange("(a p) h -> p a h", p=128)[
                                :, a, q * 1024:(q + 1) * 1024],
                        )
                        if gate_weights:
                            add_dep_helper(wd.ins, gate, sync=True,
                                           reason="after ramp")
                    w1cs.append(w1c)
                for q in range(4):
                    w2c = wp.tile([128, 8 * EMBED], BF, tag="w2s", bufs=2,
                                  name=f"w2c{rep}_{q}")
                    for j in range(8):
                        hc = q * 8 + j
                        wd = nc.sync.dma_start(
                            out=w2c[:, j * EMBED:(j + 1) * EMBED],
                            in_=w2T_d.rearrange("(a p) e -> p a e", p=128)[:, hc, :],
                        )
                        if gate_weights:
                            add_dep_helper(wd.ins, gate, sync=True,
                                           reason="after ramp")
                    w2cs.append(w2c)
                # residual rows for phase 5: load during the loop
                tgt_r = wk.tile([128, EMBED], F32, tag="tgt_r", name=f"tgtr{rep}")
                trd = nc.sync.dma_start(out=tgt_r[:], in_=tgt_rows_d[:])
                if gate_weights:
                    add_dep_helper(trd.ins, gate, sync=True, reason="after ramp")

                # ------------- phase 3: 5-step energy descent ---------------
                qbf = {}
                for th in range(2):
                    tsl = slice(th * 512, (th + 1) * 512)
                    b = wk.tile([128, 512], BF, tag=f"qbf{th}", bufs=2,
                                name=f"qbf{rep}_init{th}")
                    nc.gpsimd.tensor_copy(b[:], qT[:, tsl])
                    qbf[th] = b

                # A2A staging buffers (filled per t-half as step 5 retires)
                qfin = wk.tile([128, N_TGT], BF, tag="qfin", name=f"qfin{rep}")
                q_loc = dp.tile([N_CORES * 128, TPC], BF, name=f"qloc{rep}")
                q_ex = dp.tile([N_CORES * 128, TPC], BF, name=f"qex{rep}")

                for step in range(STEPS):
                    last_step = step == STEPS - 1
                    for th in range(2):
                        tsl = slice(th * 512, (th + 1) * 512)
                        dve_set = DVE_CHUNKS[th]
                        upd = [psB.tile([128, 512], F32, tag="psB",
                                        name=f"upd{rep}_{step}_{th}_{h}")
                               for h in range(2)]
                        exs = {}

                        def mm1_exp(kc2):
                            sc = psA.tile([128, 1024], F32, tag="psA",
                                          name=f"sc{rep}_{step}_{th}_{kc2}")
                            for h in range(2):
                                nc.tensor.matmul(
                                    sc[:, h * 512:(h + 1) * 512],
                                    KT[h * 64:(h + 1) * 64,
                                       kc2 * 128:(kc2 + 1) * 128],
                                    qbf[th][h * 64:(h + 1) * 64, :],
                                    start=True, stop=True,
                                )
                            ex = wk.tile([128, 1024], BF, tag="ex", bufs=6,
                                         name=f"ex{rep}_{step}_{th}_{kc2}")
                            if kc2 in dve_set:
                                vt = wk.tile([128, 1024], F32, tag="vexp",
                                             bufs=2,
                                             name=f"vx{rep}_{step}_{th}_{kc2}")
                                nc.vector._custom_dve(
                                    EXP_A, out=vt[:], in0=sc[:],
                                    s0=BETA / EXP_N)
                                nc.vector._custom_dve(
                                    EXP_B, out=ex[:], in0=vt[:])
                            else:
                                nc.scalar.activation(ex[:], sc[:], AF.Exp,
                                                     scale=BETA)
                            exs[kc2] = ex

                        def mm2(kc2):
                            ex = exs.pop(kc2)
                            for h in range(2):
                                nc.tensor.matmul(
                                    upd[h][:],
                                    Kaug[:, kc2 * 256 + h * 128:
                                         kc2 * 256 + (h + 1) * 128],
                                    ex[:, h * 512:(h + 1) * 512],
                                    start=(kc2 == 0), stop=(kc2 == KC - 1),
                                )

                        for kc in range(KC):
                            mm1_exp(kc)
                            if kc >= MM2_LAG:
                                mm2(kc - MM2_LAG)
                        for kc in range(KC - MM2_LAG, KC):
                            mm2(kc)

                        # tail: qT[:, tsl] += num / den
                        rec = wk.tile([128, 512], F32, tag="rec", bufs=2,
                                      name=f"rec{rep}_{step}_{th}")
                        for h in range(2):
                            nc.vector.reciprocal_approx_fast(
                                out=rec[h * 64:(h + 1) * 64, :],
                                in_=upd[h][64:128, :])
                        dq = wk.tile([128, 512], F32, tag="dq", bufs=2,
                                     name=f"dq{rep}_{step}_{th}")
                        for h in range(2):
                            nc.gpsimd.tensor_tensor(
                                dq[h * 64:(h + 1) * 64, :], upd[h][0:64, :],
                                rec[h * 64:(h + 1) * 64, :],
                                mybir.AluOpType.mult,
                            )
                        nc.gpsimd.tensor_tensor(
                            qT[:, tsl], qT[:, tsl], dq[:],
                            mybir.AluOpType.add,
                        )
                        if not last_step:
                            b = wk.tile([128, 512], BF, tag=f"qbf{th}", bufs=2,
                                        name=f"qbf{rep}_{step}_{th}")
                            nc.gpsimd.tensor_copy(b[:], qT[:, tsl])
                            qbf[th] = b
                        else:
                            # stage this t-half for the A2A immediately
                            nc.gpsimd.tensor_copy(qfin[:, tsl], qT[:, tsl])
                            nc.sync.dma_start(
                                out=q_loc[:].rearrange(
                                    "(j p) t -> p j t", p=128)[
                                        :, th * 4:(th + 1) * 4, :],
                                in_=qfin[:, tsl].rearrange(
                                    "p (j t) -> p j t", j=4),
                            )

                if skip_tail:
                    out_sb0 = wk.tile([128, EMBED], F32, tag="out_sb",
                                      name=f"outq{rep}")
                    nc.vector.tensor_copy(out_sb0[:], qT[:])
                    nc.sync.dma_start(out=out_d[:], in_=out_sb0[:])
                    return

                # ------------- phase 4: AllToAll on Q -----------------------
                # q_loc [8*128, TPC]: partition-block j holds my heads' q at
                # t-block j; after A2A, block j holds core j's heads at MY
                # t-block. bf16 halves the collective bytes.
                if no_collective:
                    nc.sync.dma_start(out=q_ex[:], in_=q_loc[:])
                else:
                    nc.gpsimd.collective_compute(
                        "AllToAll",
                        mybir.AluOpType.bypass,
                        replica_groups=[list(range(N_CORES))],
                        ins=[q_loc[:]],
                        outs=[q_ex[:]],
                    )
                qto = wk.tile([128, DC * TPC], BF, tag="qto", name=f"qto{rep}")
                for hh in range(2):
                    nc.sync.dma_start(
                        out=qto[:].rearrange("p (a t) -> p a t", a=DC)[
                            :, hh * 4:(hh + 1) * 4, :],
                        in_=q_ex[:].rearrange("(a p) t -> p a t", p=128)[
                            :, hh * 4:(hh + 1) * 4, :],
                    )

                # ------------- phase 5: Wo projection + residual ------------
                atn = psA.tile([128, 1024], F32, tag="psA", name=f"atn{rep}")
                for a in range(DC):
                    for ecol in range(2):
                        nc.tensor.matmul(
                            atn[:, ecol * 512:(ecol + 1) * 512],
                            qto[:, a * TPC:(a + 1) * TPC],
                            woT_sb[:, a * EMBED + ecol * 512:
                                   a * EMBED + (ecol + 1) * 512],
                            start=(a == 0), stop=(a == DC - 1),
                        )
                t2 = pp.tile([128, EMBED], F32, tag="t2", name=f"t2{rep}")
                t2n = wk.tile([128, EMBED], BF, tag="t2n", name=f"t2n{rep}")
                t2T = wk.tile([128, DC * TPC], BF, tag="t2T", name=f"t2T{rep}")
                # halves pipeline: add -> tanh -> 4 transposes each
                for half in range(2):
                    hsl = slice(half * 512, (half + 1) * 512)
                    nc.vector.tensor_tensor(t2[:, hsl], tgt_r[:, hsl],
                                            atn[:, hsl], mybir.AluOpType.add)
                    nc.scalar.activation(t2n[:, hsl], t2[:, hsl], AF.Tanh,
                                         scale=alphas[:, 1:2])
                    for dd in range(4):
                        d = half * 4 + dd
                        tp = psB.tile([128, 128], BF, tag="psB",
                                      name=f"t2tp{rep}_{d}")
                        nc.tensor.transpose(tp[:], t2n[:, d * 128:(d + 1) * 128],
                                            ident[:])
                        nc.gpsimd.tensor_copy(t2T[:, d * TPC:(d + 1) * TPC],
                                              tp[:])

                # ------------- phase 6: fused FFN ---------------------------
                # per hidden-chunk: HT = sum_d W1T[d,hc]^T t2T[d] (PSUM),
                # GT_hc = gelu(HT) straight from PSUM, then W2 accumulation.
                fps = psA.tile([128, 1024], F32, tag="psA", name=f"fps{rep}")
                for hc in range(HC):
                    q, j = hc // 8, hc % 8
                    w1c = w1cs[q]
                    ht = psB.tile([128, 128], F32, tag="psB",
                                  name=f"ht{rep}_{hc}")
                    for a in range(DC):
                        nc.tensor.matmul(
                            ht[:],
                            w1c[:, a * 1024 + j * 128:a * 1024 + (j + 1) * 128],
                            t2T[:, a * TPC:(a + 1) * TPC],
                            start=(a == 0), stop=(a == DC - 1),
                        )
                    gt = wk.tile([128, 128], BF, tag="gt", bufs=4,
                                 name=f"gt{rep}_{hc}")
                    nc.scalar.activation(gt[:], ht[:], GELU_FN)
                    w2c = w2cs[q]
                    for ecol in range(2):
                        nc.tensor.matmul(
                            fps[:, ecol * 512:(ecol + 1) * 512],
                            gt[:],
                            w2c[:, j * EMBED + ecol * 512:
                                j * EMBED + (ecol + 1) * 512],
                            start=(hc == 0), stop=(hc == HC - 1),
                        )
                out_sb = wk.tile([128, EMBED], F32, tag="out_sb", name=f"out{rep}")
                nc.vector.tensor_tensor(out_sb[:], t2[:], fps[:],
                                        mybir.AluOpType.add)
                nc.sync.dma_start(out=out_d[:], in_=out_sb[:])

            if loop_n > 1:
                assert no_collective and replicas == 1
                with tc.For_i(0, loop_n, 1):
                    body(0)
            else:
                for rep in range(replicas):
                    body(rep)

    nc.compile()
    return nc


def prepare_inputs(context, target, Wq, Wk, Wo, W1, W2, alpha1, alpha2):
    """Per-core host-side layout prep. Returns list of 8 in_maps."""
    bf = ml_dtypes.bfloat16
    context = np.asarray(context, np.float32)
    target = np.asarray(target, np.float32)
    ctxT = np.ascontiguousarray(context.T).astype(bf)            # [1024, 2048]
    tgtT = np.ascontiguousarray(target.T).astype(np.float32)     # [1024, 1024]
    woT = np.ascontiguousarray(np.asarray(Wo, np.float32).T).astype(bf)
    w1T = np.ascontiguousarray(np.asarray(W1, np.float32).T).astype(bf)
    w2T = np.ascontiguousarray(np.asarray(W2, np.float32).T).astype(bf)
    alphas = np.zeros((128, 2), np.float32)
    alphas[:, 0] = np.float32(np.asarray(alpha1).reshape(-1)[0])
    alphas[:, 1] = np.float32(np.asarray(alpha2).reshape(-1)[0])
    Wq = np.asarray(Wq, np.float32)
    Wk = np.asarray(Wk, np.float32)

    tgtT = tgtT.astype(bf)
    in_maps = []
    for c in range(N_CORES):
        hs = slice(c * HPC, (c + 1) * HPC)
        wq = Wq[hs].reshape(HPC * HD, EMBED)
        wkk = Wk[hs].reshape(HPC * HD, EMBED)
        wqkT = np.concatenate(
            [np.ascontiguousarray(wq.T), np.ascontiguousarray(wkk.T)], axis=1
        ).astype(bf)                                             # [1024, 256]
        in_maps.append({
            "ctxT": ctxT,
            "tgtT": tgtT,
            "tgt_rows": np.ascontiguousarray(
                target[c * TPC:(c + 1) * TPC]).astype(np.float32),
            "wqkT": wqkT,
            "woT": woT,
            "w1T": w1T,
            "w2T": w2T,
            "alphas": alphas,
        })
    return in_maps


def kernel(context, target, Wq, Wk, Wo, W1, W2, alpha1, alpha2):
    in_maps = prepare_inputs(context, target, Wq, Wk, Wo, W1, W2,
                             alpha1, alpha2)
    nc = build_kernel()
    res = run_bass_kernel_spmd(nc, in_maps, list(range(N_CORES)))
    out = np.concatenate(
        [res.results[c]["out_rows"] for c in range(N_CORES)], axis=0
    )
    return out.astype(np.float32)
